# revision 19
# baseline (speedup 1.0000x reference)
"""Trainium2 Bass kernel for HCEN forward: out = ((x.mean(axis=1)) @ W_enc.T + b_enc) @ W_out.T + b_out.

Since there is no nonlinearity between the two linear layers, they fold into
one on host: W_comb = W_out @ W_enc, b_comb = W_out @ b_enc + b_out, so the
device computes out = mean(x) @ W_comb.T + b_comb.

Sharding: data-parallel over batch. B=16 across 8 cores -> 2 batches/core.
x ships as bf16 (16 MB/core); W_comb.T as bf16 in 8 chunk DMAs interleaved
with the early x tiles on the same sync HWDGE ring (a separate-ring weight
DMA gets starved to ~58 GB/s and its completion-sem lane head-of-line blocks
the x stream when the lane is reused).

Per-core pipeline:
  warmup: ~40 tiny PE matmuls during the NEFF preamble so the HAM clock gate
    is at 2.4 GHz when the first tile lands.
  stream x in [128, QT, 1024] bf16 tiles (contiguous 16 KB per partition);
  per q-slab, two ones(=1/S)-stationary matmuls reduce 128 rows into
  psum m[1, 512] chunks. Each (batch, half) accumulation group owns a full
  PSUM bank: interleaved groups sharing one bank corrupt each other
  (observed), separate banks are safe. Trailing tiles are small (QT=2) so
  the post-stream PE tail is short.
  m -> SBUF bf16 per-batch [1, 1024] tiles (partition 0, since ACT/DVE
  cannot write at a partition offset), 8 single-shot PE transposes per batch
  ([1,128] stationary x identity[1,1]) -> mT[128, 8, 2] psum; b0's copies +
  transposes run during b1's stream. One DVE copy -> SBUF, then the combined
  layer mT.T @ W_combT -> out[2, 1024] psum, DVE bias-add, DMA out.
  Host concatenates the 8 [2, 1024] parts.
"""

import os
import sys
from contextlib import ExitStack

import ml_dtypes
import numpy as np

for _p in ("/opt/trn_rl_repo", "/root/.axon_site/_ro/trn_rl_repo"):
    if os.path.isdir(_p) and _p not in sys.path:
        sys.path.insert(0, _p)

import concourse.bass as bass  # noqa: E402
import concourse.tile as tile  # noqa: E402
from concourse import bacc, mybir  # noqa: E402
from concourse.bass_utils import run_bass_kernel_spmd  # noqa: E402


B, S, D, O = 16, 4096, 1024, 1024
NCORES = 8
BPC = B // NCORES  # batches per core
P = 128
DC = D // P
NF = 512  # matmul moving free dim (PSUM bank limit)
F32 = mybir.dt.float32
BF16 = mybir.dt.bfloat16
FP8 = mybir.dt.float8e4

# per-(batch, d-half) s-tiling: q-units of 128 rows each; the final pass
# ends with small tiles so the post-stream PE tail is short.
TILES_STD = [16, 16]
TILES_LAST = [16, 8, 4, 2, 1, 1]
QBIG = 16
NWARM = 19
DH = 2  # d-halves; chunks 0-3 of mT (and half the L layer) finish mid-stream

_CACHE = {}


def build_nc():
    if "nc" in _CACHE:
        return _CACHE["nc"]
    nc = bacc.Bacc(
        "TRN2",
        target_bir_lowering=False,
        debug=False,
        enable_asserts=False,
        num_devices=NCORES,
    )
    x_ext = nc.dram_tensor("x", [BPC, S, D], FP8, kind="ExternalInput").ap()
    wcombT_ext = nc.dram_tensor("wcombT", [D, O], BF16, kind="ExternalInput").ap()
    bcomb_ext = nc.dram_tensor("bcomb", [O], BF16, kind="ExternalInput").ap()
    out_ext = nc.dram_tensor("out", [BPC, O], F32, kind="ExternalOutput").ap()

    with ExitStack() as ctx:
        tc = ctx.enter_context(tile.TileContext(nc))
        consts = ctx.enter_context(tc.tile_pool(name="consts", bufs=1))
        wpool = ctx.enter_context(tc.tile_pool(name="wpool", bufs=1))
        xbig = ctx.enter_context(tc.tile_pool(name="xbig", bufs=8))
        xsm = ctx.enter_context(tc.tile_pool(name="xsm", bufs=2))
        spool = ctx.enter_context(tc.tile_pool(name="spool", bufs=1))
        pmp = ctx.enter_context(tc.tile_pool(name="pmp", bufs=1, space="PSUM"))
        tpp = ctx.enter_context(tc.tile_pool(name="tpp", bufs=1, space="PSUM"))
        pop = ctx.enter_context(tc.tile_pool(name="pop", bufs=1, space="PSUM"))
        pwp = ctx.enter_context(tc.tile_pool(name="pwp", bufs=1, space="PSUM"))

        ones2 = consts.tile([P, 2, P], FP8)
        nc.vector.memset(ones2[:], 1.0)  # 1/S applied at the psum->SBUF copy
        one1 = consts.tile([1, 1], F32)
        nc.vector.memset(one1[:], 1.0)
        onerow = consts.tile([1, BPC], BF16)
        nc.vector.memset(onerow[:], 1.0)

        # PE warmup: the HAM clock gate only unthrottles after one FULLY
        # busy 4096-cycle window, so the warmup must be ~4us of back-to-back
        # full-width matmuls (N=512 DoubleRow on a junk tile), not tiny ones.
        junk = consts.tile([P, 2, NF], FP8)
        nc.vector.memset(junk[:], 1.0)
        warm_ps = pwp.tile([P, NF], F32, name="warm", tag="warm")
        for _ in range(NWARM):
            nc.tensor.matmul(
                warm_ps[:], ones2[:], junk[:],
                perf_mode=mybir.MatmulPerfMode.DoubleRow,
            )

        bias_sb = consts.tile([1, O], BF16)

        # phase 1: stream x; per q-slab two ones-stationary matmuls reduce the
        # 128 rows into psum m[1, 512] halves (one PSUM bank per group).
        wcomb_sb = wpool.tile([P, DC, O], BF16)
        pm = [
            [pmp.tile([P, NF], F32, name=f"pm{b}_{h}", tag=f"pm{b}_{h}") for h in range(DH)]
            for b in range(BPC)
        ]
        out_ps = pop.tile([BPC, O], F32, name="out_ps", tag="ops")
        out_sb = spool.tile([BPC, O], F32)
        m_sb = [spool.tile([1, D], F32, name=f"m{b}") for b in range(BPC)]
        tp = tpp.tile([P, DC, BPC], F32)
        mt_sb = spool.tile([P, DC, BPC], BF16)
        wchunks = list(range(DC))  # weight chunk DMAs to interleave early

        wdone = False
        for dh in range(DH):
            dsl = slice(dh * NF, (dh + 1) * NF)
            for b in range(BPC):
                tiles = TILES_LAST if (dh == DH - 1 and b == BPC - 1) else TILES_STD
                nq_total = sum(tiles)
                qdone = 0
                for ti, qt in enumerate(tiles):
                    pool = xbig if qt == QBIG else xsm
                    xt = pool.tile([P, qt, NF], FP8, name=f"xt{qt}", tag=f"xt{qt}")
                    s0 = qdone * P
                    nc.sync.dma_start(
                        xt[:],
                        x_ext[b, s0 : s0 + P * qt, dsl].rearrange(
                            "(p q) d -> p q d", q=qt
                        ),
                    )
                    if dh == 0 and b == 0 and ti == 1:
                        nc.sync.dma_start(bias_sb[:], bcomb_ext[None, :])
                    # two weight chunks after each of the first 4 x DMAs
                    for _ in range(2):
                        if wchunks:
                            c = wchunks.pop(0)
                            nc.sync.dma_start(
                                wcomb_sb[:, c, :], wcombT_ext[c * P : (c + 1) * P, :]
                            )
                    # DoubleRow: each matmul contracts two q-slabs (256 rows)
                    for j in range(max(qt // 2, 1)):
                        q0 = 2 * j
                        pair = qt - q0 >= 2
                        if pair:
                            nc.tensor.matmul(
                                pm[b][dh][:],
                                ones2[:],
                                xt[:, q0 : q0 + 2, :],
                                start=(qdone == 0 and j == 0),
                                stop=(qdone + qt == nq_total and qt - q0 <= 2),
                                perf_mode=mybir.MatmulPerfMode.DoubleRow,
                            )
                        else:
                            nc.tensor.matmul(
                                pm[b][dh][:],
                                ones2[:, 0, :],
                                xt[:, q0, :],
                                start=(qdone == 0 and j == 0),
                                stop=(qdone + qt == nq_total and qt - q0 <= 2),
                            )
                    qdone += qt
                    # keep the HAM MID window from seeing a fully idle 3.4us
                    nc.tensor.matmul(
                        warm_ps[:], ones2[:], junk[:],
                        perf_mode=mybir.MatmulPerfMode.DoubleRow,
                    )

                # batch b's d-half done: psum m -> SBUF f32 row, 4 PE
                # transposes -> tp[:, c, b] (all mid-stream except the last)
                eng = nc.scalar if (b == 0) else nc.vector
                if b == 0:
                    nc.scalar.mul(m_sb[b][0:1, dsl], pm[b][dh][0:1, :], 1.0 / S)
                else:
                    nc.vector.tensor_scalar_mul(
                        m_sb[b][0:1, dsl], pm[b][dh][0:1, :], 1.0 / S
                    )
                for c in range(dh * 4, dh * 4 + 4):
                    nc.tensor.transpose(
                        tp[:, c, b : b + 1], m_sb[b][0:1, c * P : (c + 1) * P], one1[:]
                    )
            # both batches' d-half reduced: mt chunks -> SBUF bf16, then the
            # L-layer accumulation for these 4 contraction chunks
            csl = slice(dh * 4, dh * 4 + 4)
            nc.vector.tensor_copy(mt_sb[:, csl, :], tp[:, csl, :])
            for n in range(O // NF):
                sl = slice(n * NF, (n + 1) * NF)
                for c in range(dh * 4, dh * 4 + 4):
                    nc.tensor.matmul(
                        out_ps[:, sl],
                        mt_sb[:, c, :],
                        wcomb_sb[:, c, sl],
                        start=(c == 0),
                        stop=False,
                    )
                if dh == DH - 1:
                    # bias folded in as a K=1 rank-1 update
                    nc.tensor.matmul(
                        out_ps[:, sl], onerow[:], bias_sb[:, sl], start=False, stop=True
                    )

        # out psum -> SBUF, split across DVE and ACT
        nc.vector.tensor_copy(out_sb[:, 0:NF], out_ps[:, 0:NF])
        nc.scalar.copy(out_sb[:, NF : 2 * NF], out_ps[:, NF : 2 * NF])
        nc.sync.dma_start(out_ext[:], out_sb[:])

    nc.compile()
    _CACHE["nc"] = nc
    return nc


def make_in_maps(x, W_enc, b_enc, W_out, b_out):
    x = np.asarray(x, dtype=np.float32)
    W_enc = np.asarray(W_enc, dtype=np.float32)
    b_enc = np.asarray(b_enc, dtype=np.float32)
    W_out = np.asarray(W_out, dtype=np.float32)
    b_out = np.asarray(b_out, dtype=np.float32)

    # fold the two linear layers (no nonlinearity between them)
    wcombT = np.ascontiguousarray(
        (W_out @ W_enc).T.astype(ml_dtypes.bfloat16)
    )
    bcomb = np.ascontiguousarray((W_out @ b_enc + b_out).astype(ml_dtypes.bfloat16))
    x16 = x.astype(ml_dtypes.float8_e4m3fn)
    return [
        {
            "x": np.ascontiguousarray(x16[i * BPC : (i + 1) * BPC]),
            "wcombT": wcombT,
            "bcomb": bcomb,
        }
        for i in range(NCORES)
    ]


def gather_out(results):
    return np.ascontiguousarray(
        np.concatenate([results[i]["out"] for i in range(NCORES)], axis=0)
    )


def kernel(x, W_enc, b_enc, W_out, b_out):
    nc = build_nc()
    in_maps = make_in_maps(x, W_enc, b_enc, W_out, b_out)
    res = run_bass_kernel_spmd(nc, in_maps, list(range(NCORES)))
    return gather_out(res.results)


# revision 20
# speedup vs baseline: 1.2173x; 1.2173x over previous
"""Trainium2 Bass kernel for HCEN forward: out = ((x.mean(axis=1)) @ W_enc.T + b_enc) @ W_out.T + b_out.

Since there is no nonlinearity between the two linear layers, they fold into
one on host: W_comb = W_out @ W_enc, b_comb = W_out @ b_enc + b_out, so the
device computes out = mean(x) @ W_comb.T + b_comb.

Sharding: data-parallel over batch. B=16 across 8 cores -> 2 batches/core.
x ships as bf16 (16 MB/core); W_comb.T as bf16 in 8 chunk DMAs interleaved
with the early x tiles on the same sync HWDGE ring (a separate-ring weight
DMA gets starved to ~58 GB/s and its completion-sem lane head-of-line blocks
the x stream when the lane is reused).

Per-core pipeline:
  warmup: ~40 tiny PE matmuls during the NEFF preamble so the HAM clock gate
    is at 2.4 GHz when the first tile lands.
  stream x in [128, QT, 1024] bf16 tiles (contiguous 16 KB per partition);
  per q-slab, two ones(=1/S)-stationary matmuls reduce 128 rows into
  psum m[1, 512] chunks. Each (batch, half) accumulation group owns a full
  PSUM bank: interleaved groups sharing one bank corrupt each other
  (observed), separate banks are safe. Trailing tiles are small (QT=2) so
  the post-stream PE tail is short.
  m -> SBUF bf16 per-batch [1, 1024] tiles (partition 0, since ACT/DVE
  cannot write at a partition offset), 8 single-shot PE transposes per batch
  ([1,128] stationary x identity[1,1]) -> mT[128, 8, 2] psum; b0's copies +
  transposes run during b1's stream. One DVE copy -> SBUF, then the combined
  layer mT.T @ W_combT -> out[2, 1024] psum, DVE bias-add, DMA out.
  Host concatenates the 8 [2, 1024] parts.
"""

import os
import sys
from contextlib import ExitStack

import ml_dtypes
import numpy as np

for _p in ("/opt/trn_rl_repo", "/root/.axon_site/_ro/trn_rl_repo"):
    if os.path.isdir(_p) and _p not in sys.path:
        sys.path.insert(0, _p)

import concourse.bass as bass  # noqa: E402
import concourse.tile as tile  # noqa: E402
from concourse import bacc, mybir  # noqa: E402
from concourse.bass_utils import run_bass_kernel_spmd  # noqa: E402


B, S, D, O = 16, 4096, 1024, 1024
NCORES = 8
BPC = B // NCORES  # batches per core
P = 128
DC = D // P
NF = 512  # matmul moving free dim (PSUM bank limit)
F32 = mybir.dt.float32
BF16 = mybir.dt.bfloat16
FP8 = mybir.dt.float8e4

# per-(batch, d-half) s-tiling: q-units of 128 rows each; the final pass
# ends with small tiles so the post-stream PE tail is short.
TILES_STD = [16, 16]
TILES_LAST = [16, 8, 4, 2, 1, 1]
QBIG = 16
NWARM = 19
DH = 2  # d-halves; chunks 0-3 of mT (and half the L layer) finish mid-stream

_CACHE = {}


def build_nc():
    if "nc" in _CACHE:
        return _CACHE["nc"]
    nc = bacc.Bacc(
        "TRN2",
        target_bir_lowering=False,
        debug=False,
        enable_asserts=False,
        num_devices=NCORES,
    )
    xh_ext = [
        nc.dram_tensor(f"x{h}", [BPC, S, NF], FP8, kind="ExternalInput").ap()
        for h in range(2)
    ]
    wcombT_ext = nc.dram_tensor("wcombT", [D, O], BF16, kind="ExternalInput").ap()
    bcomb_ext = nc.dram_tensor("bcomb", [O], BF16, kind="ExternalInput").ap()
    out_ext = nc.dram_tensor("out", [BPC, O], F32, kind="ExternalOutput").ap()

    with ExitStack() as ctx:
        tc = ctx.enter_context(tile.TileContext(nc))
        consts = ctx.enter_context(tc.tile_pool(name="consts", bufs=1))
        wpool = ctx.enter_context(tc.tile_pool(name="wpool", bufs=1))
        xbig = ctx.enter_context(tc.tile_pool(name="xbig", bufs=8))
        xsm = ctx.enter_context(tc.tile_pool(name="xsm", bufs=2))
        spool = ctx.enter_context(tc.tile_pool(name="spool", bufs=1))
        pmp = ctx.enter_context(tc.tile_pool(name="pmp", bufs=1, space="PSUM"))
        tpp = ctx.enter_context(tc.tile_pool(name="tpp", bufs=1, space="PSUM"))
        pop = ctx.enter_context(tc.tile_pool(name="pop", bufs=1, space="PSUM"))
        pwp = ctx.enter_context(tc.tile_pool(name="pwp", bufs=1, space="PSUM"))

        ones2 = consts.tile([P, 2, P], FP8)
        nc.vector.memset(ones2[:], 1.0)  # 1/S applied at the psum->SBUF copy
        one1 = consts.tile([1, 1], F32)
        nc.vector.memset(one1[:], 1.0)
        onerow = consts.tile([1, BPC], BF16)
        nc.vector.memset(onerow[:], 1.0)

        # PE warmup: the HAM clock gate only unthrottles after one FULLY
        # busy 4096-cycle window, so the warmup must be ~4us of back-to-back
        # full-width matmuls (N=512 DoubleRow on a junk tile), not tiny ones.
        junk = consts.tile([P, 2, NF], FP8)
        nc.vector.memset(junk[:], 1.0)
        warm_ps = pwp.tile([P, NF], F32, name="warm", tag="warm")
        for _ in range(NWARM):
            nc.tensor.matmul(
                warm_ps[:], ones2[:], junk[:],
                perf_mode=mybir.MatmulPerfMode.DoubleRow,
            )

        bias_sb = consts.tile([1, O], BF16)

        # phase 1: stream x; per q-slab two ones-stationary matmuls reduce the
        # 128 rows into psum m[1, 512] halves (one PSUM bank per group).
        wcomb_sb = wpool.tile([P, DC, O], BF16)
        pm = [
            [pmp.tile([P, NF], F32, name=f"pm{b}_{h}", tag=f"pm{b}_{h}") for h in range(DH)]
            for b in range(BPC)
        ]
        out_ps = pop.tile([BPC, O], F32, name="out_ps", tag="ops")
        out_sb = spool.tile([BPC, O], F32)
        m_sb = [spool.tile([1, D], F32, name=f"m{b}") for b in range(BPC)]
        tp = tpp.tile([P, DC, BPC], F32)
        mt_sb = spool.tile([P, DC, BPC], BF16)
        wchunks = list(range(DC))  # weight chunk DMAs to interleave early

        wdone = False
        for dh in range(DH):
            dsl = slice(dh * NF, (dh + 1) * NF)
            for b in range(BPC):
                tiles = TILES_LAST if (dh == DH - 1 and b == BPC - 1) else TILES_STD
                nq_total = sum(tiles)
                qdone = 0
                for ti, qt in enumerate(tiles):
                    pool = xbig if qt == QBIG else xsm
                    xt = pool.tile([P, qt, NF], FP8, name=f"xt{qt}", tag=f"xt{qt}")
                    s0 = qdone * P
                    nc.sync.dma_start(
                        xt[:],
                        xh_ext[dh][b, s0 : s0 + P * qt, :].rearrange(
                            "(p q) d -> p q d", q=qt
                        ),
                    )
                    if dh == 0 and b == 0 and ti == 1:
                        nc.sync.dma_start(bias_sb[:], bcomb_ext[None, :])
                    # two weight chunks after each of the first 4 x DMAs
                    for _ in range(2):
                        if wchunks:
                            c = wchunks.pop(0)
                            nc.sync.dma_start(
                                wcomb_sb[:, c, :], wcombT_ext[c * P : (c + 1) * P, :]
                            )
                    # DoubleRow: each matmul contracts two q-slabs (256 rows)
                    for j in range(max(qt // 2, 1)):
                        q0 = 2 * j
                        pair = qt - q0 >= 2
                        if pair:
                            nc.tensor.matmul(
                                pm[b][dh][:],
                                ones2[:],
                                xt[:, q0 : q0 + 2, :],
                                start=(qdone == 0 and j == 0),
                                stop=(qdone + qt == nq_total and qt - q0 <= 2),
                                perf_mode=mybir.MatmulPerfMode.DoubleRow,
                            )
                        else:
                            nc.tensor.matmul(
                                pm[b][dh][:],
                                ones2[:, 0, :],
                                xt[:, q0, :],
                                start=(qdone == 0 and j == 0),
                                stop=(qdone + qt == nq_total and qt - q0 <= 2),
                            )
                    qdone += qt
                    # keep the HAM MID window from seeing a fully idle 3.4us
                    nc.tensor.matmul(
                        warm_ps[:], ones2[:], junk[:],
                        perf_mode=mybir.MatmulPerfMode.DoubleRow,
                    )

                # batch b's d-half done: psum m -> SBUF f32 row, 4 PE
                # transposes -> tp[:, c, b] (all mid-stream except the last)
                eng = nc.scalar if (b == 0) else nc.vector
                if b == 0:
                    nc.scalar.mul(m_sb[b][0:1, dsl], pm[b][dh][0:1, :], 1.0 / S)
                else:
                    nc.vector.tensor_scalar_mul(
                        m_sb[b][0:1, dsl], pm[b][dh][0:1, :], 1.0 / S
                    )
                for c in range(dh * 4, dh * 4 + 4):
                    nc.tensor.transpose(
                        tp[:, c, b : b + 1], m_sb[b][0:1, c * P : (c + 1) * P], one1[:]
                    )
            # both batches' d-half reduced: mt chunks -> SBUF bf16, then the
            # L-layer accumulation for these 4 contraction chunks
            csl = slice(dh * 4, dh * 4 + 4)
            nc.vector.tensor_copy(mt_sb[:, csl, :], tp[:, csl, :])
            for n in range(O // NF):
                sl = slice(n * NF, (n + 1) * NF)
                for c in range(dh * 4, dh * 4 + 4):
                    nc.tensor.matmul(
                        out_ps[:, sl],
                        mt_sb[:, c, :],
                        wcomb_sb[:, c, sl],
                        start=(c == 0),
                        stop=False,
                    )
                if dh == DH - 1:
                    # bias folded in as a K=1 rank-1 update
                    nc.tensor.matmul(
                        out_ps[:, sl], onerow[:], bias_sb[:, sl], start=False, stop=True
                    )

        # out psum -> SBUF, split across DVE and ACT
        nc.vector.tensor_copy(out_sb[:, 0:NF], out_ps[:, 0:NF])
        nc.scalar.copy(out_sb[:, NF : 2 * NF], out_ps[:, NF : 2 * NF])
        nc.sync.dma_start(out_ext[:], out_sb[:])

    nc.compile()
    _CACHE["nc"] = nc
    return nc


def make_in_maps(x, W_enc, b_enc, W_out, b_out):
    x = np.asarray(x, dtype=np.float32)
    W_enc = np.asarray(W_enc, dtype=np.float32)
    b_enc = np.asarray(b_enc, dtype=np.float32)
    W_out = np.asarray(W_out, dtype=np.float32)
    b_out = np.asarray(b_out, dtype=np.float32)

    # fold the two linear layers (no nonlinearity between them)
    wcombT = np.ascontiguousarray(
        (W_out @ W_enc).T.astype(ml_dtypes.bfloat16)
    )
    bcomb = np.ascontiguousarray((W_out @ b_enc + b_out).astype(ml_dtypes.bfloat16))
    x16 = x.astype(ml_dtypes.float8_e4m3fn)
    return [
        {
            "x0": np.ascontiguousarray(x16[i * BPC : (i + 1) * BPC, :, 0:512]),
            "x1": np.ascontiguousarray(x16[i * BPC : (i + 1) * BPC, :, 512:1024]),
            "wcombT": wcombT,
            "bcomb": bcomb,
        }
        for i in range(NCORES)
    ]


def gather_out(results):
    return np.ascontiguousarray(
        np.concatenate([results[i]["out"] for i in range(NCORES)], axis=0)
    )


def kernel(x, W_enc, b_enc, W_out, b_out):
    nc = build_nc()
    in_maps = make_in_maps(x, W_enc, b_enc, W_out, b_out)
    res = run_bass_kernel_spmd(nc, in_maps, list(range(NCORES)))
    return gather_out(res.results)
